# revision 26
# baseline (speedup 1.0000x reference)
"""Multi-head attention (B=4, S=2048, E=768, H=12) on 8 trn2 NeuronCores.

Sharding: 2-D (batch x head-half). Core c handles batch c//2, heads
(c%2)*6 .. (c%2)*6+5  (Wq/Wk/Wv column-split, Wo row-split). Each core
returns a partial O^T [768, S]; host sums the two head-halves per batch,
transposes, and adds the effective output bias (bo + bv@Wo - softmax rows
sum to 1, so V's bias contributes a constant row folded on the host).

Bias handling (exact):
  - bk only shifts every score of a query row by a constant -> softmax
    invariant -> dropped.
  - bq contributes a per-key term bq.(k@Wk) -> host folds it into the
    per-key exp bias vector (kbias) alongside the -30000 padding mask.
  - bv, bo -> host-side bo_eff = bo + bv@Wo.

Device kernel (per core), bf16 matmuls + fp32 PSUM:
  - masked keys are compacted away on host; padded keys get -30000 added
    via the exp's per-partition bias -> exp == 0.
  - scores/ctx computed transposed (S^T tiles [128 k, q]) so P^T feeds the
    context matmul directly. Score weights (K^T) are stored zero-padded to
    contraction 128 so every matmul runs in the un-tiled 128x128 PE mode
    (K=64 matmuls force the 64-row tiling mode; each mode switch drains
    the PE). The ones-broadcast matmul is zero-padded the same way.
  - the head-pair's two score tiles land in ONE 2-bank PSUM tile
    [128, 1024] f32 so a single wide Exp activation covers both heads
    (per-instruction ACT overhead ~260 cycles).
  - the attention pipeline is FLAT across (qb, p) iterations: scores are
    emitted two key-chunk steps ahead globally, so the Exp stream on the
    Scalar queue never stalls at an iteration boundary.
  - V tiles carry an appended ones column (col 64) so row 64 of the 65-row
    context accumulator is the softmax denominator.
  - normalization: denominator rows collected (DMA) into 32-aligned rows
    of collector tiles, reciprocal_approx_fast (batched), hi/lo bf16
    split, ones-outer-product broadcast matmul (exact), multiply.
  - V projection and Q projection m=1,2 run as background closures popped
    between attention steps; output projection is interleaved per
    query-block via the deferred queue.
  - DMA issue (~600ns per dma_start on a queue) is spread across the
    sync / scalar / gpsimd queues; memsets run on GpSimd to keep the
    Vector queue free for PSUM evacuations.
"""

import numpy as np
import ml_dtypes

E = 768
H = 12
D = 64
HALF = 384  # E // 2 output cols per head-half
N_CORES = 8

_CACHE = {}
_LAST = None  # last BassKernelResults (for test harness introspection)

bf16_np = ml_dtypes.bfloat16


def _build(S_q, S_pad):
    from contextlib import ExitStack
    import concourse.bass as bass
    import concourse.tile as tile
    from concourse import bacc, mybir

    bf16 = mybir.dt.bfloat16
    f32 = mybir.dt.float32
    FT = mybir.ActivationFunctionType

    NKC = S_pad // 128
    NMC = HALF // 128        # 3 proj-dim chunks (head pairs)
    NEC = E // 128           # 6 embed chunks
    QB = 512 if S_q % 512 == 0 else S_q
    NQB = S_q // QB
    NU = 6 * NQB             # normalization units (2 per (qb, p))
    NCOLL = (NU + 3) // 4    # collector tiles (4 rows each at 0/32/64/96)

    def ntiles(total, step=512):
        return [(s, min(step, total - s)) for s in range(0, total, step)]

    nc = bacc.Bacc("TRN2", target_bir_lowering=False, debug=False,
                   num_devices=N_CORES)

    qT = nc.dram_tensor("qT", [E, S_q], bf16, kind="ExternalInput").ap()
    kT = nc.dram_tensor("kT", [E, S_pad], bf16, kind="ExternalInput").ap()
    vT = nc.dram_tensor("vT", [E, S_pad], bf16, kind="ExternalInput").ap()
    wq = nc.dram_tensor("wq", [E, HALF], bf16, kind="ExternalInput").ap()
    wk = nc.dram_tensor("wk", [E, HALF], bf16, kind="ExternalInput").ap()
    wv = nc.dram_tensor("wv", [E, HALF], bf16, kind="ExternalInput").ap()
    wo = nc.dram_tensor("wo", [HALF, E], bf16, kind="ExternalInput").ap()
    kbias = nc.dram_tensor("kbias", [128, NKC], f32, kind="ExternalInput").ap()
    # bf16 partial outputs: halves the output DMA traffic; the host sums
    # the two head-half partials in fp32.
    oT = nc.dram_tensor("oT", [E, S_q], bf16, kind="ExternalOutput").ap()

    with tile.TileContext(nc) as tc, ExitStack() as ctx:
        cons = ctx.enter_context(tc.tile_pool(name="cons", bufs=1))
        wp = ctx.enter_context(tc.tile_pool(name="wp", bufs=1))
        acts = ctx.enter_context(tc.tile_pool(name="acts", bufs=1))
        pp = ctx.enter_context(tc.tile_pool(name="pp", bufs=6))
        ost = ctx.enter_context(tc.tile_pool(name="ost", bufs=4))
        nrm = ctx.enter_context(tc.tile_pool(name="nrm", bufs=1))

        # ---- constants; padded ones matrix for the broadcast matmul
        # (contraction 128 / M 128 -> stays in the 128x128 PE mode).
        kb_t = cons.tile([128, NKC], f32, tag="kb")
        ones2 = cons.tile([128, 128], bf16, tag="ones2")
        nc.sync.dma_start(kb_t[:], kbias[:])
        nc.gpsimd.memset(ones2[:], 0.0)
        nc.gpsimd.memset(ones2[0:2, 0:64], 1.0)
        # dummy activation so the exp ACT table loads off the critical path
        dmy = cons.tile([1, 8], f32, tag="dmy")
        dmy2 = cons.tile([1, 8], f32, tag="dmy2")
        nc.vector.memset(dmy[:], 0.0)
        nc.scalar.activation(dmy2[:], dmy[:], FT.Exp)
        # two persistent zeroed rhs tiles for the broadcast matmul
        # (rows 2:128 must be clean zeros; units alternate between them)
        hl_t = [nrm.tile([128, QB], bf16, tag=f"hlt{x}", name=f"hlt{x}")
                for x in range(2)]
        nc.gpsimd.memset(hl_t[0][:], 0.0)
        nc.gpsimd.memset(hl_t[1][:], 0.0)

        # ---- input DMAs spread across queues so issue time (~600ns per
        # dma_start per queue) doesn't serialize the head.
        qkv = tc.tile_pool(name="qkv", bufs=1)
        inp = qkv.__enter__()
        wq_t = [wp.tile([128, HALF], bf16, tag=f"wq{e}", name=f"wq{e}") for e in range(NEC)]
        wk_t = [wp.tile([128, HALF], bf16, tag=f"wk{e}", name=f"wk{e}") for e in range(NEC)]
        wv_t = [wp.tile([128, HALF], bf16, tag=f"wv{e}", name=f"wv{e}") for e in range(NEC)]
        wo_t = [wp.tile([128, E], bf16, tag=f"wo{m}", name=f"wo{m}") for m in range(NMC)]
        kT_t = [inp.tile([128, S_pad], bf16, tag=f"kT{e}", name=f"kTt{e}") for e in range(NEC)]
        vT_t = [inp.tile([128, S_pad], bf16, tag=f"vT{e}", name=f"vTt{e}") for e in range(NEC)]
        qT_t = [inp.tile([128, S_q], bf16, tag=f"qT{e}", name=f"qTt{e}") for e in range(NEC)]
        # single queue, priority order -- input DMA is bandwidth-bound, so
        # parallel queues would starve the critical path (kT first).
        QH = 2 * QB
        for e in range(NEC):
            nc.sync.dma_start(wk_t[e][:], wk[128 * e:128 * (e + 1), :])
            nc.sync.dma_start(kT_t[e][:], kT[128 * e:128 * (e + 1), :])
        for e in range(NEC):
            nc.sync.dma_start(wq_t[e][:], wq[128 * e:128 * (e + 1), :])
            nc.sync.dma_start(qT_t[e][:, 0:QH],
                              qT[128 * e:128 * (e + 1), 0:QH])
        for e in range(NEC):
            nc.sync.dma_start(wv_t[e][:], wv[128 * e:128 * (e + 1), :])
            nc.sync.dma_start(vT_t[e][:], vT[128 * e:128 * (e + 1), :])
        for e in range(NEC):
            nc.sync.dma_start(qT_t[e][:, QH:S_q],
                              qT[128 * e:128 * (e + 1), QH:S_q])
        for m in range(NMC):
            nc.sync.dma_start(wo_t[m][:], wo[128 * m:128 * (m + 1), :])

        # ---- activation tiles
        # kts zero-padded to contraction 128: [:, 0, :] head-A dims in rows
        # 0:64 (rows 64:128 zero), [:, 1, :] head-B dims in rows 64:128.
        # The zero halves are written inline by the K evacuations.
        kts = [acts.tile([128, 2, S_pad], bf16, tag=f"kts{m}", name=f"kts{m}")
               for m in range(NMC)]
        qts = [acts.tile([128, S_q], bf16, tag=f"qts{m}", name=f"qts{m}") for m in range(NMC)]
        vhx = [acts.tile([128, 6, 65], bf16, tag=f"vhx{j}", name=f"vhx{j}") for j in range(NKC)]
        czT = [acts.tile([128, S_q], bf16, tag=f"czT{m}", name=f"czT{m}") for m in range(NMC)]
        for m in range(NMC):
            # zero the pad halves once, off the critical Vector queue
            nc.gpsimd.memset(kts[m][64:128, 0, :], 0.0)
            nc.gpsimd.memset(kts[m][0:64, 1, :], 0.0)

        den_t = [nrm.tile([97, QB], f32, tag=f"den{t}", name=f"den{t}")
                 for t in range(NCOLL)]
        for t in range(NCOLL):
            nc.gpsimd.memset(den_t[t][:], 1.0)

        # ---- prefix projections (own PSUM pool): K (all m), Q m=0
        psp = tc.tile_pool(name="psp", bufs=1, space="PSUM")
        ps = psp.__enter__()

        def proj_k_pair(m, pair, pool, tagpfx, bufs=2):
            pjs = [pool.tile([128, 512], f32, tag=f"{tagpfx}{j}", bufs=bufs,
                             name=f"pk_{m}_{pair[0][0]}_{j}")
                   for j in range(len(pair))]
            for e in range(NEC):
                for j, (n0, nw) in enumerate(pair):
                    nc.tensor.matmul(
                        pjs[j][:, :nw],
                        wk_t[e][:, 128 * m:128 * (m + 1)],
                        kT_t[e][:, n0:n0 + nw],
                        start=(e == 0), stop=(e == NEC - 1))
            for j, (n0, nw) in enumerate(pair):
                nc.vector.tensor_copy(kts[m][0:64, 0, n0:n0 + nw],
                                      pjs[j][0:64, :nw])
                nc.vector.tensor_copy(kts[m][64:128, 1, n0:n0 + nw],
                                      pjs[j][64:128, :nw])

        def proj_q_pair(m, pair, pool, tagpfx, bufs=2):
            pjs = [pool.tile([128, 512], f32, tag=f"{tagpfx}{j}", bufs=bufs,
                             name=f"pq_{m}_{pair[0][0]}_{j}")
                   for j in range(len(pair))]
            for e in range(NEC):
                for j, (n0, nw) in enumerate(pair):
                    nc.tensor.matmul(
                        pjs[j][:, :nw],
                        wq_t[e][:, 128 * m:128 * (m + 1)],
                        qT_t[e][:, n0:n0 + nw],
                        start=(e == 0), stop=(e == NEC - 1))
            for j, (n0, nw) in enumerate(pair):
                nc.vector.tensor_copy(qts[m][:, n0:n0 + nw], pjs[j][:, :nw])

        # prefix: only K m=0 and the first Q m=0 column pair (all the first
        # scores need -- the second Q column pair would wait on the late
        # qT column-half DMA); the rest runs as background closures.
        kt_pairs = ntiles(S_pad)
        qt_pairs = ntiles(S_q)
        for i in range(0, len(kt_pairs), 2):
            proj_k_pair(0, kt_pairs[i:i + 2], ps, "pj")
        proj_q_pair(0, qt_pairs[0:2], ps, "pj")
        psp.__exit__(None, None, None)

        # ---- attention phase PSUM pool: SG 2x2 banks, CA, CB, aux x2
        psa = tc.tile_pool(name="psa", bufs=1, space="PSUM")
        ps = psa.__enter__()

        def make_vproj(i):
            js = [j for j in (i, i + 1) if j < NKC]

            def emit():
                pvs = [ps.tile([128, HALF], f32, tag=f"aux{x}", bufs=1,
                               name=f"pv{j}") for x, j in enumerate(js)]
                for e in range(NEC):
                    for x, j in enumerate(js):
                        nc.tensor.matmul(pvs[x][:],
                                         vT_t[e][:, 128 * j:128 * (j + 1)],
                                         wv_t[e][:],
                                         start=(e == 0), stop=(e == NEC - 1))
                for x, j in enumerate(js):
                    nc.gpsimd.memset(vhx[j][:, :, 64:65], 1.0)
                    nc.vector.tensor_copy(
                        vhx[j][:, :, 0:64],
                        pvs[x][:].rearrange("p (h d) -> p h d", h=6))
            return emit

        def make_qproj(m, pair):
            def emit():
                proj_q_pair(m, pair, ps, "aux", bufs=1)
            return emit

        def make_kproj(m, pair):
            def emit():
                proj_k_pair(m, pair, ps, "aux", bufs=1)
            return emit

        # order matters: ctx(kc) needs V pair kc//2, popped at 1/step;
        # K/Q projections for head-pair p must precede its scores, and the
        # second Q column pair (query blocks 2,3) can come later.
        bg = [make_vproj(i) for i in range(0, NKC, 2)]
        nv = len(bg)
        bg += [make_qproj(0, qt_pairs[2:4])]
        bg += [make_kproj(1, kt_pairs[i:i + 2])
               for i in range(0, len(kt_pairs), 2)]
        bg += [make_qproj(1, qt_pairs[0:2])]
        bg += [make_qproj(1, qt_pairs[2:4])]
        bg += [make_kproj(2, kt_pairs[i:i + 2])
               for i in range(0, len(kt_pairs), 2)]
        bg += [make_qproj(2, qt_pairs[0:2])]
        bg += [make_qproj(2, qt_pairs[2:4])]
        nk = len(range(0, len(kt_pairs), 2))

        def bg_need(qb, p):
            # pops required before scores of (qb, p): V isn't needed for
            # scores, but closures are ordered so later heads' K/Q come
            # after it; the second Q column pair only gates qb >= 2.
            if p == 0:
                return 0 if qb < 2 else nv + 1
            full = nv + 1 + (p - 1) * (nk + 2) + nk + 1
            return full if qb < 2 else full + 1
        bg_popped = [0]

        def bg_pop():
            bg.pop(0)()
            bg_popped[0] += 1

        deferred = []
        hilo = {}
        qb_done = [0] * NQB
        in_flush = [False]

        def make_group(t):
            def group():
                recq = nrm.tile([97, QB], f32, tag="recq", bufs=2,
                                name=f"recq{t}")
                nc.vector.reciprocal_approx_fast(recq[:], den_t[t][:])
                hi_t = nrm.tile([97, QB], bf16, tag="hi", bufs=2,
                                name=f"hi{t}")
                lo_t = nrm.tile([97, QB], bf16, tag="lo", bufs=2,
                                name=f"lo{t}")
                nc.vector.tensor_copy(hi_t[:], recq[:])
                nc.vector.tensor_sub(lo_t[:], recq[:], hi_t[:])
                hilo[t] = (hi_t, lo_t)
            return group

        def make_outproj(qb):
            q0 = qb * QB

            def make_pair(ec0):
                ecs = [ec0, ec0 + 1]

                def emit():
                    pos = [ps.tile([128, QB], f32, tag=f"aux{x}", bufs=1,
                                   name=f"po_{qb}_{ec}")
                           for x, ec in enumerate(ecs)]
                    for mm in range(NMC):
                        for x, ec in enumerate(ecs):
                            nc.tensor.matmul(
                                pos[x][:],
                                wo_t[mm][:, 128 * ec:128 * (ec + 1)],
                                czT[mm][:, q0:q0 + QB],
                                start=(mm == 0), stop=(mm == NMC - 1))
                    for x, ec in enumerate(ecs):
                        ot = ost.tile([128, QB], bf16, tag="ot",
                                      name=f"ot{qb}_{ec}")
                        if in_flush[0] and x == 1:
                            nc.scalar.copy(ot[:], pos[x][:])
                        else:
                            nc.vector.tensor_copy(ot[:], pos[x][:])
                        nc.gpsimd.dma_start(
                            oT[128 * ec:128 * (ec + 1), q0:q0 + QB], ot[:])
                return emit
            return [make_pair(ec0) for ec0 in range(0, NEC, 2)]

        def make_unit(u, cs, qb, m, half):
            t, r = divmod(u, 4)

            def unit():
                hi_t, lo_t = hilo[t]
                hl = hl_t[u % 2]
                nc.sync.dma_start(hl[0:1, :], hi_t[32 * r:32 * r + 1, :])
                nc.sync.dma_start(hl[1:2, :], lo_t[32 * r:32 * r + 1, :])
                bcp = ps.tile([128, QB], f32, tag="aux0", bufs=1,
                              name=f"bcp{u}")
                nc.tensor.matmul(bcp[:], ones2[:], hl[:],
                                 start=True, stop=True)
                nc.vector.tensor_mul(
                    czT[m][64 * half:64 * (half + 1), qb * QB:(qb + 1) * QB],
                    cs[0:64, :], bcp[0:64, :])
                qb_done[qb] += 1
                if qb_done[qb] == 6:
                    deferred.extend(make_outproj(qb))
            return unit

        group_units = {}
        ucount = [0]

        def evac(qb, m, half, C):
            u = ucount[0]
            ucount[0] += 1
            cs = nrm.tile([65, QB], f32, tag="cs", bufs=5, name=f"cs{u}")
            nc.vector.tensor_copy(cs[:], C[0:65, :])
            t, r = divmod(u, 4)
            nc.sync.dma_start(den_t[t][32 * r:32 * r + 1, :], cs[64:65, :])
            group_units.setdefault(t, []).append(
                make_unit(u, cs, qb, m, half))
            if r == 3 or u == NU - 1:
                deferred.append(make_group(t))
                deferred.extend(group_units.pop(t))

        # ---- flat attention pipeline over (qb, p, kc) steps
        its = [(qb, p) for qb in range(NQB) for p in range(NMC)]
        nsteps = len(its) * NKC
        SG_tiles = {}
        CAB = {}
        pend_cs = []

        def sc_emit(s):
            qb, p = its[s // NKC]
            kc = s % NKC
            q0 = qb * QB
            while bg and bg_popped[0] < bg_need(qb, p):
                bg_pop()
            SG = ps.tile([128, 2 * QB], f32, tag="SG", bufs=2,
                         name=f"SG{s}")
            nc.tensor.matmul(SG[:, 0:QB],
                             kts[p][:, 0, 128 * kc:128 * (kc + 1)],
                             qts[p][:, q0:q0 + QB],
                             start=True, stop=True)
            nc.tensor.matmul(SG[:, QB:2 * QB],
                             kts[p][:, 1, 128 * kc:128 * (kc + 1)],
                             qts[p][:, q0:q0 + QB],
                             start=True, stop=True)
            SG_tiles[s] = SG

        for s in range(nsteps):
            qb, p = its[s // NKC]
            kc = s % NKC
            if kc == 0:
                CA = ps.tile([65, QB], f32, tag="CA", name=f"CA{qb}_{p}")
                CB = ps.tile([65, QB], f32, tag="CB", name=f"CB{qb}_{p}")
                CAB[s // NKC] = (CA, CB)
                for pc in pend_cs:
                    evac(*pc)
                pend_cs = []
                if s == 0:
                    sc_emit(0)
                    sc_emit(1)
            if s + 2 < nsteps:
                sc_emit(s + 2)
            if bg:
                bg_pop()
            elif deferred:
                deferred.pop(0)()
            CA, CB = CAB[s // NKC]
            PG = pp.tile([128, 2 * QB], bf16, tag="PG", name=f"PG{s}")
            nc.scalar.activation(PG[:], SG_tiles.pop(s)[:], FT.Exp,
                                 bias=kb_t[:, kc:kc + 1], scale=1.0)
            nc.tensor.matmul(CA[:], vhx[kc][:, 2 * p, :], PG[:, 0:QB],
                             start=(kc == 0), stop=(kc == NKC - 1))
            nc.tensor.matmul(CB[:], vhx[kc][:, 2 * p + 1, :],
                             PG[:, QB:2 * QB],
                             start=(kc == 0), stop=(kc == NKC - 1))
            if kc == NKC - 1:
                pend_cs = [(qb, p, 0, CA), (qb, p, 1, CB)]

        for pc in pend_cs:
            evac(*pc)
        while bg:
            bg_pop()
        in_flush[0] = True
        while deferred:
            deferred.pop(0)()
        psa.__exit__(None, None, None)
        qkv.__exit__(None, None, None)

    nc.compile()
    return nc


def _numpy_fallback(q, k, v, mask, Wq, bq, Wk, bk, Wv, bv, Wo, bo):
    B, Sq, _ = q.shape
    qh = (q @ Wq + bq).reshape(B, Sq, H, D).transpose(0, 2, 1, 3)
    kh = (k @ Wk + bk).reshape(B, -1, H, D).transpose(0, 2, 1, 3)
    vh = (v @ Wv + bv).reshape(B, -1, H, D).transpose(0, 2, 1, 3)
    s = np.einsum("bhqd,bhkd->bhqk", qh, kh) / np.sqrt(np.float32(D))
    s = s + np.where(mask == 0, np.float32(-1e9), np.float32(0))[:, None, None, :]
    s = s - s.max(-1, keepdims=True)
    w = np.exp(s)
    w = w / w.sum(-1, keepdims=True)
    ctx = np.einsum("bhqk,bhkd->bqhd", w, vh).reshape(B, Sq, E)
    return (ctx @ Wo + bo).astype(np.float32)


def kernel(q, k, v, mask, Wq, bq, Wk, bk, Wv, bv, Wo, bo):
    global _LAST
    q = np.asarray(q, np.float32)
    k = np.asarray(k, np.float32)
    v = np.asarray(v, np.float32)
    mask = np.asarray(mask)
    Wq = np.asarray(Wq, np.float32)
    bq = np.asarray(bq, np.float32)
    Wk = np.asarray(Wk, np.float32)
    bk = np.asarray(bk, np.float32)
    Wv = np.asarray(Wv, np.float32)
    bv = np.asarray(bv, np.float32)
    Wo = np.asarray(Wo, np.float32)
    bo = np.asarray(bo, np.float32)

    B, S_q, _ = q.shape
    idxs = [np.flatnonzero(mask[b]) for b in range(B)]
    ns = [len(ix) for ix in idxs]
    if min(ns) == 0 or B * 2 != N_CORES or S_q % 512 != 0:
        return _numpy_fallback(q, k, v, mask, Wq, bq, Wk, bk, Wv, bv, Wo, bo)

    S_pad = max(128, ((max(ns) + 127) // 128) * 128)
    NKC = S_pad // 128

    key = (S_q, S_pad)
    if key not in _CACHE:
        _CACHE[key] = _build(S_q, S_pad)
    nc = _CACHE[key]

    scale = np.float32(1.0 / np.sqrt(D))
    bq_nonzero = bool(np.any(bq))
    in_maps = []
    for c in range(N_CORES):
        b, j = divmod(c, 2)
        cols = slice(j * HALF, (j + 1) * HALF)
        kc_ = np.zeros((S_pad, E), np.float32)
        kc_[:ns[b]] = k[b][idxs[b]]
        vc_ = np.zeros((S_pad, E), np.float32)
        vc_[:ns[b]] = v[b][idxs[b]]
        kb_vec = np.zeros(S_pad, np.float32)
        kb_vec[ns[b]:] = -30000.0
        if bq_nonzero:
            kb_vec[:ns[b]] += scale * (
                kc_[:ns[b]] @ (Wk[:, cols] @ bq[cols])
                + bk[cols] @ bq[cols])
        in_maps.append({
            "qT": np.ascontiguousarray(q[b].T).astype(bf16_np),
            "kT": np.ascontiguousarray(kc_.T).astype(bf16_np),
            "vT": np.ascontiguousarray(vc_.T).astype(bf16_np),
            "wq": (Wq[:, cols] * scale).astype(bf16_np),
            "wk": np.ascontiguousarray(Wk[:, cols]).astype(bf16_np),
            "wv": np.ascontiguousarray(Wv[:, cols]).astype(bf16_np),
            "wo": np.ascontiguousarray(Wo[cols, :]).astype(bf16_np),
            "kbias": np.ascontiguousarray(kb_vec.reshape(NKC, 128).T),
        })

    from concourse.bass_utils import run_bass_kernel_spmd
    res = run_bass_kernel_spmd(nc, in_maps, list(range(N_CORES)))
    _LAST = res

    bo_eff = bo + bv @ Wo
    out = np.empty((B, S_q, E), np.float32)
    for b in range(B):
        out[b] = (res.results[2 * b]["oT"].astype(np.float32)
                  + res.results[2 * b + 1]["oT"].astype(np.float32)).T
        out[b] += bo_eff
    return out


# revision 35
# speedup vs baseline: 1.0289x; 1.0289x over previous
"""Multi-head attention (B=4, S=2048, E=768, H=12) on 8 trn2 NeuronCores.

Sharding: 2-D (batch x head-half). Core c handles batch c//2, heads
(c%2)*6 .. (c%2)*6+5  (Wq/Wk/Wv column-split, Wo row-split). Each core
returns a partial O^T [768, S]; host sums the two head-halves per batch,
transposes, and adds the effective output bias (bo + bv@Wo - softmax rows
sum to 1, so V's bias contributes a constant row folded on the host).

Bias handling (exact):
  - bk only shifts every score of a query row by a constant -> softmax
    invariant -> dropped.
  - bq contributes a per-key term bq.(k@Wk) -> host folds it into the
    per-key exp bias vector (kbias) alongside the -30000 padding mask.
  - bv, bo -> host-side bo_eff = bo + bv@Wo.

Device kernel (per core), bf16 matmuls + fp32 PSUM:
  - masked keys are compacted away on host; padded keys get -30000 added
    via the exp's per-partition bias -> exp == 0.
  - scores/ctx computed transposed (S^T tiles [128 k, q]) so P^T feeds the
    context matmul directly. Score weights (K^T) are stored zero-padded to
    contraction 128 so every matmul runs in the un-tiled 128x128 PE mode
    (K=64 matmuls force the 64-row tiling mode; each mode switch drains
    the PE). The ones-broadcast matmul is zero-padded the same way.
  - the head-pair's two score tiles land in ONE 2-bank PSUM tile
    [128, 1024] f32 so a single wide Exp activation covers both heads
    (per-instruction ACT overhead ~260 cycles).
  - the attention pipeline is FLAT across (qb, p) iterations: scores are
    emitted two key-chunk steps ahead globally, so the Exp stream on the
    Scalar queue never stalls at an iteration boundary.
  - V tiles carry an appended ones column (col 64) so row 64 of the 65-row
    context accumulator is the softmax denominator.
  - normalization: denominator rows collected (DMA) into 32-aligned rows
    of collector tiles, reciprocal_approx_fast (batched), hi/lo bf16
    split, ones-outer-product broadcast matmul (exact), multiply.
  - V projection and Q projection m=1,2 run as background closures popped
    between attention steps; output projection is interleaved per
    query-block via the deferred queue.
  - DMA issue (~600ns per dma_start on a queue) is spread across the
    sync / scalar / gpsimd queues; memsets run on GpSimd to keep the
    Vector queue free for PSUM evacuations.
"""

import numpy as np
import ml_dtypes

E = 768
H = 12
D = 64
HALF = 384  # E // 2 output cols per head-half
N_CORES = 8

_CACHE = {}
_LAST = None  # last BassKernelResults (for test harness introspection)

bf16_np = ml_dtypes.bfloat16


def _build(S_q, S_pad):
    from contextlib import ExitStack
    import concourse.bass as bass
    import concourse.tile as tile
    from concourse import bacc, mybir

    bf16 = mybir.dt.bfloat16
    f32 = mybir.dt.float32
    FT = mybir.ActivationFunctionType

    NKC = S_pad // 128
    NMC = HALF // 128        # 3 proj-dim chunks (head pairs)
    NEC = E // 128           # 6 embed chunks
    QB = 512 if S_q % 512 == 0 else S_q
    NQB = S_q // QB
    NU = 6 * NQB             # normalization units (2 per (qb, p))
    NCOLL = NU // 2          # collector tiles (2 rows each at 0/32)

    def ntiles(total, step=512):
        return [(s, min(step, total - s)) for s in range(0, total, step)]

    nc = bacc.Bacc("TRN2", target_bir_lowering=False, debug=False,
                   num_devices=N_CORES)

    qT = nc.dram_tensor("qT", [E, S_q], bf16, kind="ExternalInput").ap()
    kT = nc.dram_tensor("kT", [E, S_pad], bf16, kind="ExternalInput").ap()
    vT = nc.dram_tensor("vT", [E, S_pad], bf16, kind="ExternalInput").ap()
    wq = nc.dram_tensor("wq", [E, HALF], bf16, kind="ExternalInput").ap()
    wk = nc.dram_tensor("wk", [E, HALF], bf16, kind="ExternalInput").ap()
    wv = nc.dram_tensor("wv", [E, HALF], bf16, kind="ExternalInput").ap()
    wo = nc.dram_tensor("wo", [HALF, E], bf16, kind="ExternalInput").ap()
    kbias = nc.dram_tensor("kbias", [128, NKC], f32, kind="ExternalInput").ap()
    # bf16 partial outputs: halves the output DMA traffic; the host sums
    # the two head-half partials in fp32.
    oT = nc.dram_tensor("oT", [E, S_q], bf16, kind="ExternalOutput").ap()

    with tile.TileContext(nc) as tc, ExitStack() as ctx:
        cons = ctx.enter_context(tc.tile_pool(name="cons", bufs=1))
        wp = ctx.enter_context(tc.tile_pool(name="wp", bufs=1))
        acts = ctx.enter_context(tc.tile_pool(name="acts", bufs=1))
        pp = ctx.enter_context(tc.tile_pool(name="pp", bufs=6))
        ost = ctx.enter_context(tc.tile_pool(name="ost", bufs=4))
        nrm = ctx.enter_context(tc.tile_pool(name="nrm", bufs=1))

        # ---- constants; padded ones matrix for the broadcast matmul
        # (contraction 128 / M 128 -> stays in the 128x128 PE mode).
        kb_t = cons.tile([128, NKC], f32, tag="kb")
        # ones4: broadcast matrix for a PAIR of normalization units --
        # hl rows 0,1 (hi+lo of unit A) -> bcp rows 0:64, hl rows 32,33
        # (unit B) -> bcp rows 64:128 (row 32 start: engine APs must begin
        # at partition 0/32/64/96). Zero-padded to 128x128 PE mode.
        ones4 = cons.tile([128, 128], bf16, tag="ones4")
        nc.sync.dma_start(kb_t[:], kbias[:])
        nc.gpsimd.memset(ones4[:], 0.0)
        nc.gpsimd.memset(ones4[0:2, 0:64], 1.0)
        nc.gpsimd.memset(ones4[32:34, 64:128], 1.0)
        # dummy activation so the exp ACT table loads off the critical path
        dmy = cons.tile([1, 8], f32, tag="dmy")
        dmy2 = cons.tile([1, 8], f32, tag="dmy2")
        nc.vector.memset(dmy[:], 0.0)
        nc.scalar.activation(dmy2[:], dmy[:], FT.Exp)
        # two persistent zeroed rhs tiles for the broadcast matmul
        # (rows 2:128 must be clean zeros; units alternate between them)
        hl_t = [nrm.tile([128, QB], bf16, tag=f"hlt{x}", name=f"hlt{x}")
                for x in range(2)]
        nc.gpsimd.memset(hl_t[0][:], 0.0)
        nc.gpsimd.memset(hl_t[1][:], 0.0)

        # ---- input DMAs spread across queues so issue time (~600ns per
        # dma_start per queue) doesn't serialize the head.
        qkv = tc.tile_pool(name="qkv", bufs=1)
        inp = qkv.__enter__()
        wq_t = [wp.tile([128, HALF], bf16, tag=f"wq{e}", name=f"wq{e}") for e in range(NEC)]
        wk_t = [wp.tile([128, HALF], bf16, tag=f"wk{e}", name=f"wk{e}") for e in range(NEC)]
        wv_t = [wp.tile([128, HALF], bf16, tag=f"wv{e}", name=f"wv{e}") for e in range(NEC)]
        wo_t = [wp.tile([128, E], bf16, tag=f"wo{m}", name=f"wo{m}") for m in range(NMC)]
        kT_t = [inp.tile([128, S_pad], bf16, tag=f"kT{e}", name=f"kTt{e}") for e in range(NEC)]
        vT_t = [inp.tile([128, S_pad], bf16, tag=f"vT{e}", name=f"vTt{e}") for e in range(NEC)]
        qT_t = [inp.tile([128, S_q], bf16, tag=f"qT{e}", name=f"qTt{e}") for e in range(NEC)]
        # single queue, priority order -- input DMA is bandwidth-bound, so
        # parallel queues would starve the critical path (kT first).
        QH = 2 * QB
        for e in range(NEC):
            nc.sync.dma_start(wk_t[e][:], wk[128 * e:128 * (e + 1), :])
            nc.sync.dma_start(kT_t[e][:], kT[128 * e:128 * (e + 1), :])
        for e in range(NEC):
            nc.sync.dma_start(wq_t[e][:], wq[128 * e:128 * (e + 1), :])
            nc.sync.dma_start(qT_t[e][:, 0:QH],
                              qT[128 * e:128 * (e + 1), 0:QH])
        for e in range(NEC):
            nc.sync.dma_start(wv_t[e][:], wv[128 * e:128 * (e + 1), :])
            nc.sync.dma_start(vT_t[e][:], vT[128 * e:128 * (e + 1), :])
        for e in range(NEC):
            nc.sync.dma_start(qT_t[e][:, QH:S_q],
                              qT[128 * e:128 * (e + 1), QH:S_q])
        for m in range(NMC):
            nc.sync.dma_start(wo_t[m][:], wo[128 * m:128 * (m + 1), :])

        # ---- activation tiles
        # kts zero-padded to contraction 128: [:, 0, :] head-A dims in rows
        # 0:64 (rows 64:128 zero), [:, 1, :] head-B dims in rows 64:128.
        # The zero halves are written inline by the K evacuations.
        kts = [acts.tile([128, 2, S_pad], bf16, tag=f"kts{m}", name=f"kts{m}")
               for m in range(NMC)]
        qts = [acts.tile([128, S_q], bf16, tag=f"qts{m}", name=f"qts{m}") for m in range(NMC)]
        vhx = [acts.tile([128, 6, 65], bf16, tag=f"vhx{j}", name=f"vhx{j}") for j in range(NKC)]
        czT = [acts.tile([128, S_q], bf16, tag=f"czT{m}", name=f"czT{m}") for m in range(NMC)]
        for m in range(NMC):
            # zero the pad halves once, off the critical Vector queue
            nc.gpsimd.memset(kts[m][64:128, 0, :], 0.0)
            nc.gpsimd.memset(kts[m][0:64, 1, :], 0.0)

        den_t = [nrm.tile([33, QB], f32, tag=f"den{t}", name=f"den{t}")
                 for t in range(NCOLL)]
        for t in range(NCOLL):
            nc.gpsimd.memset(den_t[t][:], 1.0)

        # ---- prefix projections (own PSUM pool): K (all m), Q m=0
        psp = tc.tile_pool(name="psp", bufs=1, space="PSUM")
        ps = psp.__enter__()

        def proj_k_pair(m, pair, pool, tagpfx, bufs=2):
            pjs = [pool.tile([128, 512], f32, tag=f"{tagpfx}{j}", bufs=bufs,
                             name=f"pk_{m}_{pair[0][0]}_{j}")
                   for j in range(len(pair))]
            for e in range(NEC):
                for j, (n0, nw) in enumerate(pair):
                    nc.tensor.matmul(
                        pjs[j][:, :nw],
                        wk_t[e][:, 128 * m:128 * (m + 1)],
                        kT_t[e][:, n0:n0 + nw],
                        start=(e == 0), stop=(e == NEC - 1))
            for j, (n0, nw) in enumerate(pair):
                nc.vector.tensor_copy(kts[m][0:64, 0, n0:n0 + nw],
                                      pjs[j][0:64, :nw])
                nc.vector.tensor_copy(kts[m][64:128, 1, n0:n0 + nw],
                                      pjs[j][64:128, :nw])

        def proj_q_pair(m, pair, pool, tagpfx, bufs=2):
            pjs = [pool.tile([128, 512], f32, tag=f"{tagpfx}{j}", bufs=bufs,
                             name=f"pq_{m}_{pair[0][0]}_{j}")
                   for j in range(len(pair))]
            for e in range(NEC):
                for j, (n0, nw) in enumerate(pair):
                    nc.tensor.matmul(
                        pjs[j][:, :nw],
                        wq_t[e][:, 128 * m:128 * (m + 1)],
                        qT_t[e][:, n0:n0 + nw],
                        start=(e == 0), stop=(e == NEC - 1))
            for j, (n0, nw) in enumerate(pair):
                nc.vector.tensor_copy(qts[m][:, n0:n0 + nw], pjs[j][:, :nw])

        # prefix: only K m=0 and the first Q m=0 column pair (all the first
        # scores need -- the second Q column pair would wait on the late
        # qT column-half DMA); the rest runs as background closures.
        kt_pairs = ntiles(S_pad)
        qt_pairs = ntiles(S_q)
        for i in range(0, len(kt_pairs), 2):
            proj_k_pair(0, kt_pairs[i:i + 2], ps, "pj")
        proj_q_pair(0, qt_pairs[0:2], ps, "pj")
        psp.__exit__(None, None, None)

        # ---- attention phase PSUM pool: SG 2x2 banks, CA, CB, aux x2
        psa = tc.tile_pool(name="psa", bufs=1, space="PSUM")
        ps = psa.__enter__()

        def make_vproj(i):
            js = [j for j in (i, i + 1) if j < NKC]

            def emit():
                pvs = [ps.tile([128, HALF], f32, tag=f"aux{x}", bufs=1,
                               name=f"pv{j}") for x, j in enumerate(js)]
                for e in range(NEC):
                    for x, j in enumerate(js):
                        nc.tensor.matmul(pvs[x][:],
                                         vT_t[e][:, 128 * j:128 * (j + 1)],
                                         wv_t[e][:],
                                         start=(e == 0), stop=(e == NEC - 1))
                for x, j in enumerate(js):
                    nc.gpsimd.memset(vhx[j][:, :, 64:65], 1.0)
                    nc.vector.tensor_copy(
                        vhx[j][:, :, 0:64],
                        pvs[x][:].rearrange("p (h d) -> p h d", h=6))
            return emit

        def make_qproj(m, pair):
            def emit():
                proj_q_pair(m, pair, ps, "aux", bufs=1)
            return emit

        def make_kproj(m, pair):
            def emit():
                proj_k_pair(m, pair, ps, "aux", bufs=1)
            return emit

        # order matters: ctx(kc) needs V pair kc//2, popped at 1/step;
        # K/Q projections for head-pair p must precede its scores, and the
        # second Q column pair (query blocks 2,3) can come later.
        bg = [make_vproj(i) for i in range(0, NKC, 2)]
        nv = len(bg)
        bg += [make_qproj(0, qt_pairs[2:4])]
        bg += [make_kproj(1, kt_pairs[i:i + 2])
               for i in range(0, len(kt_pairs), 2)]
        bg += [make_qproj(1, qt_pairs[0:2])]
        bg += [make_qproj(1, qt_pairs[2:4])]
        bg += [make_kproj(2, kt_pairs[i:i + 2])
               for i in range(0, len(kt_pairs), 2)]
        bg += [make_qproj(2, qt_pairs[0:2])]
        bg += [make_qproj(2, qt_pairs[2:4])]
        nk = len(range(0, len(kt_pairs), 2))

        def bg_need(qb, p):
            # pops required before scores of (qb, p): V isn't needed for
            # scores, but closures are ordered so later heads' K/Q come
            # after it; the second Q column pair only gates qb >= 2.
            if p == 0:
                return 0 if qb < 2 else nv + 1
            full = nv + 1 + (p - 1) * (nk + 2) + nk + 1
            return full if qb < 2 else full + 1
        bg_popped = [0]

        def bg_pop():
            bg.pop(0)()
            bg_popped[0] += 1

        deferred = []
        hilo = {}
        qb_done = [0] * NQB
        in_flush = [False]

        def make_group(t):
            def group():
                recq = nrm.tile([33, QB], f32, tag="recq", bufs=2,
                                name=f"recq{t}")
                nc.vector.reciprocal_approx_fast(recq[:], den_t[t][:])
                hi_t = nrm.tile([33, QB], bf16, tag="hi", bufs=2,
                                name=f"hi{t}")
                lo_t = nrm.tile([33, QB], bf16, tag="lo", bufs=2,
                                name=f"lo{t}")
                nc.vector.tensor_copy(hi_t[:], recq[:])
                nc.vector.tensor_sub(lo_t[:], recq[:], hi_t[:])
                hilo[t] = (hi_t, lo_t)
            return group

        def make_outproj(qb):
            q0 = qb * QB

            def make_pair(ec0):
                ecs = [ec0, ec0 + 1]

                def emit():
                    pos = [ps.tile([128, QB], f32, tag=f"aux{x}", bufs=1,
                                   name=f"po_{qb}_{ec}")
                           for x, ec in enumerate(ecs)]
                    for mm in range(NMC):
                        for x, ec in enumerate(ecs):
                            nc.tensor.matmul(
                                pos[x][:],
                                wo_t[mm][:, 128 * ec:128 * (ec + 1)],
                                czT[mm][:, q0:q0 + QB],
                                start=(mm == 0), stop=(mm == NMC - 1))
                    for x, ec in enumerate(ecs):
                        ot = ost.tile([128, QB], bf16, tag="ot",
                                      name=f"ot{qb}_{ec}")
                        if in_flush[0] and x == 1:
                            nc.scalar.copy(ot[:], pos[x][:])
                        else:
                            nc.vector.tensor_copy(ot[:], pos[x][:])
                        nc.gpsimd.dma_start(
                            oT[128 * ec:128 * (ec + 1), q0:q0 + QB], ot[:])
                return emit
            return [make_pair(ec0) for ec0 in range(0, NEC, 2)]

        def unit_done(qb):
            qb_done[qb] += 1
            if qb_done[qb] == 6:
                deferred.extend(make_outproj(qb))

        def make_unit_pair(u0, items):
            # items = [(cs, qb, m, half) for units u0 (A) and u0+1 (B)]
            t = u0 // 2

            def unit():
                hi_t, lo_t = hilo[t]
                hl = hl_t[(u0 // 2) % 2]
                for x in range(2):
                    nc.sync.dma_start(hl[32 * x:32 * x + 1, :],
                                      hi_t[32 * x:32 * x + 1, :])
                    nc.sync.dma_start(hl[32 * x + 1:32 * x + 2, :],
                                      lo_t[32 * x:32 * x + 1, :])
                bcp = ps.tile([128, QB], f32, tag="aux0", bufs=1,
                              name=f"bcp{u0}")
                nc.tensor.matmul(bcp[:], ones4[:], hl[:],
                                 start=True, stop=True)
                for x, (cs, qb, m, half) in enumerate(items):
                    nc.vector.tensor_mul(
                        czT[m][64 * half:64 * (half + 1),
                               qb * QB:(qb + 1) * QB],
                        cs[0:64, :], bcp[64 * x:64 * (x + 1), :])
                    unit_done(qb)
            return unit

        group_units = {}
        ucount = [0]

        def evac(qb, m, half, C):
            u = ucount[0]
            ucount[0] += 1
            cs = nrm.tile([65, QB], f32, tag="cs", bufs=5, name=f"cs{u}")
            nc.vector.tensor_copy(cs[:], C[0:65, :])
            t, r = divmod(u, 2)
            nc.sync.dma_start(den_t[t][32 * r:32 * r + 1, :], cs[64:65, :])
            group_units.setdefault(t, []).append((cs, qb, m, half))
            if r == 1:
                deferred.append(make_group(t))
                deferred.append(make_unit_pair(2 * t, group_units.pop(t)))

        # ---- flat attention pipeline over (qb, p, kc) steps
        its = [(qb, p) for qb in range(NQB) for p in range(NMC)]
        nsteps = len(its) * NKC
        SG_tiles = {}
        CAB = {}
        pend_cs = []

        def sc_emit(s):
            qb, p = its[s // NKC]
            kc = s % NKC
            q0 = qb * QB
            while bg and bg_popped[0] < bg_need(qb, p):
                bg_pop()
            SG = ps.tile([128, 2 * QB], f32, tag="SG", bufs=2,
                         name=f"SG{s}")
            nc.tensor.matmul(SG[:, 0:QB],
                             kts[p][:, 0, 128 * kc:128 * (kc + 1)],
                             qts[p][:, q0:q0 + QB],
                             start=True, stop=True)
            nc.tensor.matmul(SG[:, QB:2 * QB],
                             kts[p][:, 1, 128 * kc:128 * (kc + 1)],
                             qts[p][:, q0:q0 + QB],
                             start=True, stop=True)
            SG_tiles[s] = SG

        for s in range(nsteps):
            qb, p = its[s // NKC]
            kc = s % NKC
            if kc == 0:
                CA = ps.tile([65, QB], f32, tag="CA", name=f"CA{qb}_{p}")
                CB = ps.tile([65, QB], f32, tag="CB", name=f"CB{qb}_{p}")
                CAB[s // NKC] = (CA, CB)
                for pc in pend_cs:
                    evac(*pc)
                pend_cs = []
                if s == 0:
                    sc_emit(0)
                    sc_emit(1)
            if s + 2 < nsteps:
                sc_emit(s + 2)
            if bg:
                bg_pop()
            elif deferred:
                deferred.pop(0)()
            CA, CB = CAB[s // NKC]
            PG = pp.tile([128, 2 * QB], bf16, tag="PG", name=f"PG{s}")
            nc.scalar.activation(PG[:], SG_tiles.pop(s)[:], FT.Exp,
                                 bias=kb_t[:, kc:kc + 1], scale=1.0)
            nc.tensor.matmul(CA[:], vhx[kc][:, 2 * p, :], PG[:, 0:QB],
                             start=(kc == 0), stop=(kc == NKC - 1))
            nc.tensor.matmul(CB[:], vhx[kc][:, 2 * p + 1, :],
                             PG[:, QB:2 * QB],
                             start=(kc == 0), stop=(kc == NKC - 1))
            if kc == NKC - 1:
                pend_cs = [(qb, p, 0, CA), (qb, p, 1, CB)]

        for pc in pend_cs:
            evac(*pc)
        while bg:
            bg_pop()
        in_flush[0] = True
        while deferred:
            deferred.pop(0)()
        psa.__exit__(None, None, None)
        qkv.__exit__(None, None, None)

    nc.compile()
    return nc


def _numpy_fallback(q, k, v, mask, Wq, bq, Wk, bk, Wv, bv, Wo, bo):
    B, Sq, _ = q.shape
    qh = (q @ Wq + bq).reshape(B, Sq, H, D).transpose(0, 2, 1, 3)
    kh = (k @ Wk + bk).reshape(B, -1, H, D).transpose(0, 2, 1, 3)
    vh = (v @ Wv + bv).reshape(B, -1, H, D).transpose(0, 2, 1, 3)
    s = np.einsum("bhqd,bhkd->bhqk", qh, kh) / np.sqrt(np.float32(D))
    s = s + np.where(mask == 0, np.float32(-1e9), np.float32(0))[:, None, None, :]
    s = s - s.max(-1, keepdims=True)
    w = np.exp(s)
    w = w / w.sum(-1, keepdims=True)
    ctx = np.einsum("bhqk,bhkd->bqhd", w, vh).reshape(B, Sq, E)
    return (ctx @ Wo + bo).astype(np.float32)


def kernel(q, k, v, mask, Wq, bq, Wk, bk, Wv, bv, Wo, bo):
    global _LAST
    q = np.asarray(q, np.float32)
    k = np.asarray(k, np.float32)
    v = np.asarray(v, np.float32)
    mask = np.asarray(mask)
    Wq = np.asarray(Wq, np.float32)
    bq = np.asarray(bq, np.float32)
    Wk = np.asarray(Wk, np.float32)
    bk = np.asarray(bk, np.float32)
    Wv = np.asarray(Wv, np.float32)
    bv = np.asarray(bv, np.float32)
    Wo = np.asarray(Wo, np.float32)
    bo = np.asarray(bo, np.float32)

    B, S_q, _ = q.shape
    idxs = [np.flatnonzero(mask[b]) for b in range(B)]
    ns = [len(ix) for ix in idxs]
    if min(ns) == 0 or B * 2 != N_CORES or S_q % 512 != 0:
        return _numpy_fallback(q, k, v, mask, Wq, bq, Wk, bk, Wv, bv, Wo, bo)

    S_pad = max(128, ((max(ns) + 127) // 128) * 128)
    NKC = S_pad // 128

    key = (S_q, S_pad)
    if key not in _CACHE:
        _CACHE[key] = _build(S_q, S_pad)
    nc = _CACHE[key]

    scale = np.float32(1.0 / np.sqrt(D))
    bq_nonzero = bool(np.any(bq))
    in_maps = []
    for c in range(N_CORES):
        b, j = divmod(c, 2)
        cols = slice(j * HALF, (j + 1) * HALF)
        kc_ = np.zeros((S_pad, E), np.float32)
        kc_[:ns[b]] = k[b][idxs[b]]
        vc_ = np.zeros((S_pad, E), np.float32)
        vc_[:ns[b]] = v[b][idxs[b]]
        kb_vec = np.zeros(S_pad, np.float32)
        kb_vec[ns[b]:] = -30000.0
        if bq_nonzero:
            kb_vec[:ns[b]] += scale * (
                kc_[:ns[b]] @ (Wk[:, cols] @ bq[cols])
                + bk[cols] @ bq[cols])
        in_maps.append({
            "qT": np.ascontiguousarray(q[b].T).astype(bf16_np),
            "kT": np.ascontiguousarray(kc_.T).astype(bf16_np),
            "vT": np.ascontiguousarray(vc_.T).astype(bf16_np),
            "wq": (Wq[:, cols] * scale).astype(bf16_np),
            "wk": np.ascontiguousarray(Wk[:, cols]).astype(bf16_np),
            "wv": np.ascontiguousarray(Wv[:, cols]).astype(bf16_np),
            "wo": np.ascontiguousarray(Wo[cols, :]).astype(bf16_np),
            "kbias": np.ascontiguousarray(kb_vec.reshape(NKC, 128).T),
        })

    from concourse.bass_utils import run_bass_kernel_spmd
    res = run_bass_kernel_spmd(nc, in_maps, list(range(N_CORES)))
    _LAST = res

    bo_eff = bo + bv @ Wo
    out = np.empty((B, S_q, E), np.float32)
    for b in range(B):
        out[b] = (res.results[2 * b]["oT"].astype(np.float32)
                  + res.results[2 * b + 1]["oT"].astype(np.float32)).T
        out[b] += bo_eff
    return out
